# revision 19
# baseline (speedup 1.0000x reference)
"""CaptioningRNN (LSTM + spatial attention + vocab loss) on 8 Trainium2 cores.

Strategy:
 - The three big parallel matmul groups (CNN-feature projection, x@Wx,
   vocab logits + logsumexp) are sharded 8 ways.
 - The sequential LSTM is sharded over the 4H gate dimension: core c owns a
   128-wide slice of each gate (aligned with hidden slice c). Each step ends
   with a fused AllGather carrying (a) the core's transposed h-slice (feeds
   the next step's matmuls on every core) and (b) partial attention scores
   for the core's hidden slice (summed on every core after the gather).
 - Attention:  scores_partial = ones^T @ (A^T_slice * h^T_slice)  on PE,
   attn @ Wattn is eliminated via B[n,p,:] = A[n,:,p] @ Wattn precompute and
   a per-position multiply tree (DVE tensor_scalar 4x mode + Act muls).
 - All spatial tensors are position-major (p, n) so DVE elementwise ops hit
   the 2x packed mode; the attention-score payload is transposed to batch-
   major once on the send side (cheap) so receive DMAs stay 64B-granular.
 - Gates use tanh-only activations (sigmoid(x) = 0.5*tanh(x/2)+0.5 with the
   affine fixup on DVE) so every in-loop Act op lives in the exp_and_others
   table set: zero activation-table reloads inside the recurrence.
 - x_t @ Wx + b is accumulated into the same PSUM tile as h @ Wh at the
   start of each step (before the h-gather lands) - fills the AllGather
   window with PE work and kills the separate xwx precompute pass.
 - Loss: logsumexp partials per vocab shard (no max subtraction - logits are
   O(3)), label logits via host-gathered W_vocab columns, one final AllGather,
   then a replicated tiny reduction. b_vocab[y] mask term added on host.

KREPS (env, default 1): benchmark-only in-NEFF repetition of the whole
computation; used to measure per-iteration HW time as a wall-clock slope
(amortizes the ~100ms axon dispatch overhead). Grading path uses KREPS=1.
"""
import sys, os, time

sys.path.insert(0, "/opt/trn_rl_repo")

import numpy as np
import ml_dtypes

import concourse.bass as bass
import concourse.bacc as bacc
import concourse.mybir as mybir
import concourse.tile as tile
import concourse.bass_isa as bass_isa
from concourse.bass_utils import run_bass_kernel_spmd


BF16 = ml_dtypes.bfloat16
F32 = mybir.dt.float32
BF = mybir.dt.bfloat16
U8 = mybir.dt.uint8

NCORES = 8
N = 128          # batch
TCAP = 31
CIN = 1280
WD = 512         # embed dim
H = 1024         # hidden
V = 10000        # vocab
P = 16           # spatial positions
HS = H // NCORES      # 128 hidden slice
SC = 4 * HS           # 512 a-columns per core
VS = V // NCORES      # 1250 vocab slice
AluOp = mybir.AluOpType
Act = mybir.ActivationFunctionType

HT_BYTES = 128 * 128 * 2           # h^T slice, bf16
SC_BYTES = 2048 * 2                # partial scores, bf16
PAY = HT_BYTES + SC_BYTES          # per-step AG payload bytes

N_ACT_MULS = 6                     # chain muls offloaded to Act engine


def build(T):
    KREPS = int(os.environ.get("KREPS", "1"))
    nc = bacc.Bacc("TRN2", target_bir_lowering=False, debug=False,
                   num_devices=NCORES)
    NT = T * N

    def din(name, shape, dt):
        return nc.dram_tensor(name, shape, dt, kind="ExternalInput").ap()

    imgsT = din("imgsT", [CIN, P * N], BF)          # position-major (p, n)
    wprojS = din("wprojS", [CIN, HS], BF)
    bprojS = din("bprojS", [HS, 1], F32)
    xembT = din("xembT", [WD, NT], BF)
    WxS = din("WxS", [WD, SC], BF)
    bS1 = din("bS1", [1, SC], BF)
    WhS = din("WhS", [H, SC], BF)
    WattnS = din("WattnS", [H, SC], BF)
    wvoc = din("wvoc", [H, VS], BF)
    bvoc = din("bvoc", [1, VS], BF)
    wyT = din("wyT", [T, 128, H], BF)               # inner (k, n)
    mask_f = din("mask_f", [N, T], F32)
    ident = din("ident", [128, 128], BF)
    ones32 = din("ones32", [128, 32], BF)  # value 1/32 (scores scale)
    ones1 = din("ones1", [128, 1], BF)
    onesrow = din("onesrow", [1, 128], BF)

    loss_out = nc.dram_tensor("loss", [1, 1], F32, kind="ExternalOutput").ap()

    VCH = [(0, 512), (512, 512), (1024, VS - 1024)]  # vocab chunks
    ZB = N * T * 4
    rg = [list(range(NCORES))]

    with tile.TileContext(nc) as tc:
        with (
            tc.tile_pool(name="dram", bufs=1, space="DRAM") as dram,
            tc.tile_pool(name="persist", bufs=1) as pp,
            tc.tile_pool(name="work", bufs=3) as wp,
            tc.tile_pool(name="psB", bufs=1, space="PSUM") as psB,
        ):
            # ---------- persistent SBUF ----------
            atm = pp.tile([128, P, N], BF, name="atm")        # A^T my-slice (p, n)
            at_all = pp.tile([128, 8, P, N], BF, name="at_all")
            b_sb = pp.tile([128, P, SC], BF, name="b_sb")
            whs_sb = pp.tile([128, 8, SC], BF, name="whs_sb")
            wvoc_sb = pp.tile([128, 8, VS], BF, name="wvoc_sb")
            bvoc_sb = pp.tile([1, VS], BF, name="bvoc_sb")
            zc_sb = pp.tile([128, T, 3], F32, name="zc_sb")

            c_sb = pp.tile([128, 128], F32, name="c_sb")
            ident_sb = pp.tile([128, 128], BF, name="ident_sb")
            ones32_sb = pp.tile([128, 32], BF, name="ones32_sb")
            ones1_sb = pp.tile([128, 1], BF, name="ones1_sb")
            onesrow_sb = pp.tile([1, 128], BF, name="onesrow_sb")
            bproj_sb = pp.tile([HS, 1], F32, name="bproj_sb")
            mask_sb = pp.tile([N, T], F32, name="mask_sb")
            wproj_sb = pp.tile([128, 10, 128], BF, name="wproj_sb")
            watt_sb = pp.tile([128, 8, SC], BF, name="watt_sb")
            wxs_sb = pp.tile([128, 4, SC], BF, name="wxs_sb")
            bs1_sb = pp.tile([1, SC], BF, name="bs1_sb")

            for dst, src in [(ident_sb, ident), (ones32_sb, ones32),
                             (ones1_sb, ones1), (onesrow_sb, onesrow),
                             (bproj_sb, bprojS), (mask_sb, mask_f),
                             (bvoc_sb, bvoc), (bs1_sb, bS1)]:
                nc.sync.dma_start(dst[:], src[:])
            for kt in range(8):
                nc.sync.dma_start(whs_sb[:, kt, :], WhS[bass.ts(kt, 128), :])
                nc.sync.dma_start(wvoc_sb[:, kt, :], wvoc[bass.ts(kt, 128), :])
                nc.sync.dma_start(watt_sb[:, kt, :], WattnS[bass.ts(kt, 128), :])
            for kt in range(10):
                nc.sync.dma_start(wproj_sb[:, kt, :], wprojS[bass.ts(kt, 128), :])
            for kt in range(4):
                nc.sync.dma_start(wxs_sb[:, kt, :], WxS[bass.ts(kt, 128), :])

            for rep in range(KREPS):
                R = f"_r{rep}"

                # ---------- P1: projection  A^T[my hslice, (p, n)] ----------
                ps_a = psB.tile([128, P * N], F32, name=f"ps_a{R}", tag="ps_big")
                for kt in range(10):
                    imgs_kt = wp.tile([128, P * N], BF, name=f"imgs{kt}{R}",
                                      tag="imgs_kt", bufs=2)
                    nc.sync.dma_start(imgs_kt[:], imgsT[bass.ts(kt, 128), :])
                    for ch in range(4):
                        nc.tensor.matmul(ps_a[:, bass.ts(ch, 512)],
                                         wproj_sb[:, kt, :],
                                         imgs_kt[:, bass.ts(ch, 512)],
                                         start=(kt == 0), stop=(kt == 9))
                nc.scalar.activation(atm[:].rearrange("p q n -> p (q n)"),
                                     ps_a[:], Act.Identity, bias=bproj_sb[:])
                # h0^T my slice = mean over positions (strided reduce over p)
                h0t_f = pp.tile([128, 128], F32, name=f"h0t_f{R}", tag="h0t_f")
                nc.vector.tensor_reduce(h0t_f[:], atm[:].transpose([0, 2, 1]),
                                        mybir.AxisListType.X, AluOp.add)
                nc.scalar.mul(h0t_f[:], h0t_f[:], 1.0 / P)
                h0t_b = pp.tile([128, 128], BF, name=f"h0t_b{R}", tag="h0t_b")
                nc.scalar.copy(h0t_b[:], h0t_f[:])
                # c0 = h0 (batch-major my hidden slice)
                ps_tr0 = psB.tile([128, 128], BF, name=f"ps_tr{R}", tag="ps_scr",
                                  bufs=1)
                nc.tensor.transpose(ps_tr0[:], h0t_b[:], ident_sb[:])
                nc.scalar.copy(c_sb[:], ps_tr0[:])

                # scores partial + payload + AG.
                # ones-trick: lhsT = (1/32)*ones [128,32] -> each 32-row group
                # of the PSUM gets the column sums; rows {0,32,64,96} hold the
                # 4 chunks of the 2048-wide partial-score vector (p-major).
                def scores_and_ag(step, hT_bf):
                    e_sb = wp.tile([128, P, N], BF, name=f"e_{step}{R}",
                                   tag="e_sb", bufs=2)
                    nc.vector.tensor_mul(
                        e_sb[:], atm[:],
                        hT_bf[:].unsqueeze(1).broadcast_to([128, P, 128]))
                    ps_sc = psB.tile([128, 512], F32, name=f"ps_sc{step}{R}",
                                     tag="ps_scr", bufs=1)
                    ev = e_sb[:].rearrange("p q n -> p (q n)")
                    for ch in range(4):
                        nc.tensor.matmul(ps_sc[32 * ch:32 * (ch + 1), :],
                                         ones32_sb[:],
                                         ev[:, bass.ts(ch, 512)],
                                         start=True, stop=True,
                                         tile_position=(0, 32 * ch))
                    sc_out = wp.tile([128, 512], BF, name=f"sco{step}{R}",
                                     tag="sc_out", bufs=2)
                    nc.scalar.copy(sc_out[:], ps_sc[:])
                    pay = dram.tile([PAY], U8, name=f"pay{step}{R}")
                    nc.sync.dma_start(
                        pay[0:HT_BYTES].rearrange("(p b) -> p b", p=128),
                        hT_bf[:].bitcast(mybir.dt.uint8))
                    # transpose p-major partial scores to batch-major so the
                    # 8-core receive DMA reads 32B runs per (n, c2)
                    nc.sync.dma_start(
                        pay[HT_BYTES:PAY].bitcast(BF)
                        .rearrange("(n ph pl) -> ph pl n", n=128, ph=4),
                        sc_out[0:128:32, :].rearrange("r (pl n) -> r pl n", pl=4))
                    gat = dram.tile([NCORES, PAY], U8, name=f"gat{step}{R}")
                    nc.gpsimd.collective_compute(
                        "AllGather", AluOp.bypass, replica_groups=rg,
                        ins=[pay.opt()], outs=[gat.opt()])
                    return gat

                gat = scores_and_ag(0, h0t_b)

                # ---------- at_all: gather A^T from all cores ----------
                pay_a = dram.tile([128, P * N], BF, name=f"pay_a{R}")
                nc.sync.dma_start(pay_a[:], atm[:].rearrange("p q n -> p (q n)"))
                gat_a = dram.tile([NCORES * 128, P * N], BF, name=f"gat_a{R}")
                nc.gpsimd.collective_compute(
                    "AllGather", AluOp.bypass, replica_groups=rg,
                    ins=[pay_a.opt()], outs=[gat_a.opt()])
                gav = gat_a[:].rearrange("(c p) f -> c p f", c=NCORES)
                for kt in range(8):
                    nc.sync.dma_start(
                        at_all[:, kt, :, :].rearrange("p q n -> p (q n)"),
                        gav[kt])

                # ---------- B precompute: B[n,p,:] = A[n,:,p] @ WattnS ----------
                for p in range(P):
                    ps_b = psB.tile([128, SC], F32, name=f"ps_b{p}{R}",
                                    tag="ps_mm", bufs=2)
                    for kt in range(8):
                        nc.tensor.matmul(ps_b[:], at_all[:, kt, p, :],
                                         watt_sb[:, kt, :],
                                         start=(kt == 0), stop=(kt == 7))
                    nc.scalar.copy(b_sb[:, p, :], ps_b[:])

                # ---------- recurrence ----------
                pay2 = dram.tile([N * T * 4 + T * 128 * 4], U8, name=f"pay2{R}")

                def vocab_row(trow, hT_sb):
                    # logsumexp partials + label logits for hs row `trow`
                    for ci, (off, ln) in enumerate(VCH):
                        ps_v = psB.tile([128, 512], F32,
                                        name=f"ps_v{trow}_{ci}{R}",
                                        tag="ps_mm", bufs=2)
                        for kt in range(8):
                            nc.tensor.matmul(ps_v[:, :ln], hT_sb[:, kt, :],
                                             wvoc_sb[:, kt, off:off + ln],
                                             start=(kt == 0), stop=False)
                        nc.tensor.matmul(ps_v[:, :ln], onesrow_sb[:],
                                         bvoc_sb[:, off:off + ln],
                                         start=False, stop=True)
                        ex_scr = wp.tile([128, 512], BF, name=f"ex{trow}_{ci}{R}",
                                         tag="ex_scr", bufs=2)
                        nc.scalar.activation(ex_scr[:, :ln], ps_v[:, :ln], Act.Exp,
                                             accum_out=zc_sb[:, trow, ci:ci + 1])
                    # label logit: ey[p, k, n] = hT[p, k, n] * wy[p, k, n]
                    wyt_sb = wp.tile([128, H], BF, name=f"wyt{trow}{R}",
                                     tag="wyt_sb", bufs=2)
                    nc.sync.dma_start(wyt_sb[:], wyT[trow])
                    ey_sb = wp.tile([128, 8, 128], BF, name=f"ey{trow}{R}",
                                    tag="ey_sb", bufs=2)
                    nc.vector.tensor_mul(
                        ey_sb[:], hT_sb[:],
                        wyt_sb[:].rearrange("p (k n) -> p k n", k=8))
                    ps_ll = psB.tile([1, 1024], F32, name=f"ps_ll{trow}{R}",
                                     tag="ps_big", bufs=1)
                    eyv = ey_sb[:].rearrange("p k n -> p (k n)")
                    for ch in range(2):
                        nc.tensor.matmul(ps_ll[:, bass.ts(ch, 512)], ones1_sb[:],
                                         eyv[:, bass.ts(ch, 512)],
                                         start=True, stop=True)
                    ll_t = wp.tile([1, 128], F32, name=f"ll_t{trow}{R}",
                                   tag="ll_t", bufs=2)
                    nc.vector.tensor_reduce(
                        ll_t[:],
                        ps_ll[:].rearrange("o (k n) -> o n k", k=8),
                        mybir.AxisListType.X, AluOp.add)
                    nc.sync.dma_start(
                        pay2[ZB:].bitcast(F32)
                        .rearrange("(n t) -> t n", n=128)[trow:trow + 1, :],
                        ll_t[:])

                for t in range(T):
                    # gates psum: x@Wx + b first (PE fills the AG window)
                    xt_sb = wp.tile([128, 4, 128], BF, name=f"xt{t}{R}",
                                    tag="xt_sb", bufs=2)
                    for kt in range(4):
                        nc.sync.dma_start(xt_sb[:, kt, :],
                                          xembT[bass.ts(kt, 128), bass.ts(t, 128)])
                    ps_h = psB.tile([128, SC], F32, name=f"ps_h{t}{R}",
                                    tag="ps_h", bufs=1)
                    for kt in range(4):
                        nc.tensor.matmul(ps_h[:], xt_sb[:, kt, :], wxs_sb[:, kt, :],
                                         start=(kt == 0), stop=False)
                    nc.tensor.matmul(ps_h[:], onesrow_sb[:], bs1_sb[:],
                                     start=False, stop=False)
                    # fused receive of the gathered payload, split across the
                    # two HWDGE queues (SP + Act) to halve the issue latency
                    hT_sb = wp.tile([128, 8, 128], BF, name=f"hT{t}{R}",
                                    tag="hT_sb", bufs=3)
                    gat_ht = gat[:, 0:HT_BYTES].bitcast(BF).rearrange(
                        "c (p b) -> p c b", p=128)
                    nc.sync.dma_start(hT_sb[:, 0:4, :], gat_ht[:, 0:4, :])
                    nc.scalar.dma_start(hT_sb[:, 4:8, :], gat_ht[:, 4:8, :])
                    scr_sb = wp.tile([128, 8, P], BF, name=f"sc{t}{R}",
                                     tag="scr_sb")
                    nc.sync.dma_start(
                        scr_sb[:],
                        gat[:, HT_BYTES:PAY].bitcast(BF)
                        .rearrange("c (n q) -> n c q", n=128))
                    # softmax weights (no max subtraction: |scores| < ~10)
                    ssum = wp.tile([128, P], F32, name=f"ssum{t}{R}", tag="ssum")
                    nc.vector.tensor_reduce(ssum[:],
                                            scr_sb[:].transpose([0, 2, 1]),
                                            mybir.AxisListType.X, AluOp.add)
                    e_w = wp.tile([128, P], F32, name=f"ew{t}{R}", tag="e_w")
                    zs = wp.tile([128, 1], F32, name=f"zs{t}{R}", tag="zs")
                    nc.scalar.activation(e_w[:], ssum[:], Act.Exp, accum_out=zs[:])
                    rz = wp.tile([128, 1], F32, name=f"rz{t}{R}", tag="rz")
                    nc.vector.reciprocal(rz[:], zs[:])
                    w_sb = wp.tile([128, P], F32, name=f"w{t}{R}", tag="w_sb")
                    nc.vector.tensor_scalar_mul(w_sb[:], e_w[:], rz[:])
                    # h @ WhS into the same psum
                    for kt in range(8):
                        nc.tensor.matmul(ps_h[:], hT_sb[:, kt, :], whs_sb[:, kt, :],
                                         start=False, stop=(kt == 7))
                    # attention multiply tree: a = ps_h + sum_p w_p * B_p.
                    # Streaming binary-counter reduction keeps <=2 live tiles
                    # per level so small tag rotations suffice. Last
                    # N_ACT_MULS positions run their mul on the Act engine.
                    stack = []  # (level, tile)
                    ns = 0
                    for p in range(P):
                        mt = wp.tile([128, SC], BF, name=f"m{t}_{p}{R}",
                                     tag=f"m{p % 4}", bufs=2)
                        if p >= P - N_ACT_MULS:
                            nc.scalar.mul(mt[:], b_sb[:, p, :], w_sb[:, p:p + 1])
                        else:
                            nc.vector.tensor_scalar_mul(mt[:], b_sb[:, p, :],
                                                        w_sb[:, p:p + 1])
                        stack.append((0, mt))
                        while len(stack) >= 2 and stack[-1][0] == stack[-2][0]:
                            lv, rt = stack.pop()
                            _, lt = stack.pop()
                            if lv == 3:       # final pair -> merge with psum
                                break
                            s = wp.tile([128, SC], BF, name=f"s{t}_{ns}{R}",
                                        tag=f"s{lv}_{ns % 2}", bufs=2)
                            nc.vector.tensor_add(s[:], lt[:], rt[:])
                            stack.append((lv + 1, s))
                            ns += 1
                    assert len(stack) == 0 and lv == 3
                    d0 = wp.tile([128, SC], BF, name=f"d0{t}{R}", tag="d0")
                    nc.vector.tensor_add(d0[:], lt[:], rt[:])
                    a_sb = wp.tile([128, SC], BF, name=f"a{t}{R}", tag="a_sb")
                    nc.vector.tensor_add(a_sb[:], d0[:], ps_h[:])
                    # gates [i | f | o | g]; sigmoid(x) = 0.5*tanh(x/2)+0.5
                    ti = wp.tile([128, 128], BF, name=f"ti{t}{R}", tag="ti")
                    tf = wp.tile([128, 128], BF, name=f"tf{t}{R}", tag="tf")
                    to = wp.tile([128, 128], BF, name=f"to{t}{R}", tag="to")
                    tg = wp.tile([128, 128], BF, name=f"tg{t}{R}", tag="tg")
                    nc.scalar.activation(ti[:], a_sb[:, 0:128], Act.Tanh, scale=0.5)
                    nc.scalar.activation(tf[:], a_sb[:, 128:256], Act.Tanh, scale=0.5)
                    nc.scalar.activation(to[:], a_sb[:, 256:384], Act.Tanh, scale=0.5)
                    nc.scalar.activation(tg[:], a_sb[:, 384:512], Act.Tanh)
                    si = wp.tile([128, 128], BF, name=f"si{t}{R}", tag="si")
                    sf = wp.tile([128, 128], BF, name=f"sf{t}{R}", tag="sf")
                    so = wp.tile([128, 128], BF, name=f"so{t}{R}", tag="so")
                    nc.vector.tensor_scalar(si[:], ti[:], 0.5, 0.5,
                                            AluOp.mult, AluOp.add)
                    nc.vector.tensor_scalar(sf[:], tf[:], 0.5, 0.5,
                                            AluOp.mult, AluOp.add)
                    nc.vector.tensor_scalar(so[:], to[:], 0.5, 0.5,
                                            AluOp.mult, AluOp.add)
                    fc = wp.tile([128, 128], F32, name=f"fc{t}{R}", tag="fc")
                    ig = wp.tile([128, 128], F32, name=f"ig{t}{R}", tag="ig")
                    nc.vector.tensor_mul(fc[:], sf[:], c_sb[:])
                    nc.vector.tensor_mul(ig[:], si[:], tg[:])
                    nc.vector.tensor_add(c_sb[:], fc[:], ig[:])
                    tc_t = wp.tile([128, 128], BF, name=f"tc{t}{R}", tag="tc_t")
                    nc.scalar.activation(tc_t[:], c_sb[:], Act.Tanh)
                    h_sl = wp.tile([128, 128], BF, name=f"hsl{t}{R}", tag="h_sl")
                    nc.vector.tensor_mul(h_sl[:], so[:], tc_t[:])
                    # transpose h slice
                    ps_tr = psB.tile([128, 128], BF, name=f"ps_tr{t}{R}",
                                     tag="ps_scr", bufs=1)
                    nc.tensor.transpose(ps_tr[:], h_sl[:], ident_sb[:])
                    hT_c = wp.tile([128, 128], BF, name=f"hTc{t}{R}", tag="hT_c")
                    nc.scalar.copy(hT_c[:], ps_tr[:])
                    # scores partial for h_{t+1} + AllGather
                    gat = scores_and_ag(t + 1, hT_c)
                    # vocab row t-1: its PE matmuls and Act exps run inside
                    # the AllGather wait window just dispatched above
                    if t > 0:
                        vocab_row(t - 1, hT_sb)

                # tail: vocab for last row (h_T from final AG)
                hT_last = wp.tile([128, 8, 128], BF, name=f"hT_last{R}",
                                  tag="hT_sb")
                nc.sync.dma_start(
                    hT_last[:],
                    gat[:, 0:HT_BYTES].bitcast(BF)
                    .rearrange("c (p b) -> p c b", p=128))
                vocab_row(T - 1, hT_last)

                # ---------- final loss ----------
                zfin = wp.tile([128, T], F32, name=f"zfin{R}", tag="zfin")
                nc.vector.tensor_reduce(zfin[:], zc_sb[:], mybir.AxisListType.X,
                                        AluOp.add)
                nc.sync.dma_start(
                    pay2[0:ZB].bitcast(F32).rearrange("(p b) -> p b", p=128),
                    zfin[:])
                gat2 = dram.tile([NCORES, N * T * 4 + T * 128 * 4], U8,
                                 name=f"gat2{R}")
                nc.gpsimd.collective_compute(
                    "AllGather", AluOp.bypass, replica_groups=rg,
                    ins=[pay2.opt()], outs=[gat2.opt()])
                zg = wp.tile([128, T, 8], F32, name=f"zg{R}", tag="zg")
                lg = wp.tile([128, T, 8], F32, name=f"lg{R}", tag="lg")
                for c2 in range(8):
                    nc.sync.dma_start(
                        zg[:, :, c2],
                        gat2[c2, 0:ZB].bitcast(F32).rearrange("(n q) -> n q", n=128))
                    nc.sync.dma_start(
                        lg[:, :, c2],
                        gat2[c2, ZB:].bitcast(F32)
                        .rearrange("(n q) -> n q", n=128))
                zred = wp.tile([128, T], F32, name=f"zred{R}", tag="zred")
                llred = wp.tile([128, T], F32, name=f"llred{R}", tag="llred")
                nc.vector.tensor_reduce(zred[:], zg[:], mybir.AxisListType.X,
                                        AluOp.add)
                nc.vector.tensor_reduce(llred[:], lg[:], mybir.AxisListType.X,
                                        AluOp.add)
                lse = wp.tile([128, T], F32, name=f"lse{R}", tag="lse")
                nc.scalar.activation(lse[:], zred[:], Act.Ln)
                diff = wp.tile([128, T], F32, name=f"diff{R}", tag="diff")
                nc.vector.tensor_sub(diff[:], lse[:], llred[:])
                nc.vector.tensor_mul(diff[:], diff[:], mask_sb[:])
                per_n = wp.tile([128, 1], F32, name=f"per_n{R}", tag="per_n")
                nc.vector.tensor_reduce(per_n[:], diff[:], mybir.AxisListType.X,
                                        AluOp.add)
                pn_red = wp.tile([128, 1], F32, name=f"pn_red{R}", tag="pn_red")
                nc.gpsimd.partition_all_reduce(pn_red[:], per_n[:], 128,
                                               bass_isa.ReduceOp.add)
                loss_sb = wp.tile([1, 1], F32, name=f"loss_sb{R}", tag="loss_sb")
                nc.scalar.mul(loss_sb[:], pn_red[0:1, :], 1.0 / N)
                nc.sync.dma_start(loss_out[:], loss_sb[:])

    nc.compile()
    return nc


def host_prep(inputs, T):
    """Build the 8 per-core input maps (all numpy)."""
    g = {k: np.asarray(v) for k, v in inputs.items()}
    images, captions = g["images"], g["captions"]
    W_embed, W_proj, b_proj = g["W_embed"], g["W_proj"], g["b_proj"]
    Wx, Wh, Wattn, b = g["Wx"], g["Wh"], g["Wattn"], g["b"]
    W_vocab, b_vocab = g["W_vocab"], g["b_vocab"]

    cap = np.asarray(captions)
    cap_in = cap[:, :T]
    cap_out = cap[:, 1:T + 1]
    x_emb = W_embed[cap_in]                      # [N, T, WD]
    xembT = np.ascontiguousarray(
        x_emb.transpose(2, 1, 0).reshape(WD, T * N)).astype(BF16)
    imgsT = np.ascontiguousarray(                # position-major (cin, p, n)
        images.reshape(N, CIN, P).transpose(1, 2, 0).reshape(CIN, P * N)
    ).astype(BF16)
    mask = (cap_out != 0).astype(np.float32)     # [N, T]
    ident = np.eye(128, dtype=BF16)
    ones32 = np.full((128, 32), 1.0 / 32.0, dtype=BF16)
    ones1 = np.ones((128, 1), dtype=BF16)
    onesrow = np.ones((1, 128), dtype=BF16)

    # label weight vectors, [H, N, T] -> per t: [hl, (kt, n)]
    wy = W_vocab[:, cap_out]                     # [H, N, T]
    wy_t = wy.reshape(8, 128, N, T).transpose(3, 1, 0, 2)  # [T, hl, kt, n]

    in_maps = []
    for c in range(NCORES):
        hsl = slice(128 * c, 128 * (c + 1))
        idx = np.concatenate([g4 * H + 128 * c + np.arange(128)
                              for g4 in range(4)])
        vsl = slice(VS * c, VS * (c + 1))
        wyc = wy_t.copy()
        nm = np.zeros(N, dtype=wy_t.dtype)
        nm[16 * c:16 * (c + 1)] = 1
        wyc *= nm[None, None, None, :]
        in_maps.append({
            "imgsT": imgsT,
            "wprojS": np.ascontiguousarray(W_proj[:, hsl]).astype(BF16),
            "bprojS": np.ascontiguousarray(b_proj[hsl, None]).astype(np.float32),
            "xembT": xembT,
            "WxS": np.ascontiguousarray(Wx[:, idx]).astype(BF16),
            "bS1": np.ascontiguousarray(b[None, idx]).astype(BF16),
            "WhS": np.ascontiguousarray(Wh[:, idx]).astype(BF16),
            "WattnS": np.ascontiguousarray(Wattn[:, idx]).astype(BF16),
            "wvoc": np.ascontiguousarray(W_vocab[:, vsl]).astype(BF16),
            "bvoc": np.ascontiguousarray(b_vocab[None, vsl]).astype(BF16),
            "wyT": np.ascontiguousarray(
                wyc.reshape(T, 128, H)).astype(BF16),
            "mask_f": mask,
            "ident": ident,
            "ones32": ones32,
            "ones1": ones1,
            "onesrow": onesrow,
        })
    host_by = float(np.sum(mask.astype(np.float64) *
                           np.asarray(b_vocab, np.float64)[cap_out]) / N)
    return in_maps, host_by


_CACHE = {}


def _get_built(T):
    if T not in _CACHE:
        _CACHE[T] = build(T)
    return _CACHE[T]


def run(inputs, T=30):
    nc = _get_built(T)
    in_maps, host_by = host_prep(inputs, T)
    res = run_bass_kernel_spmd(nc, in_maps, core_ids=list(range(NCORES)))
    dev_loss = float(res.results[0]["loss"][0, 0])
    return np.float32(dev_loss - host_by)


def kernel(**inputs) -> np.ndarray:
    return run(inputs, T=30)
